# revision 2
# baseline (speedup 1.0000x reference)
"""Multi-head self-attention on 8 TRN2 NeuronCores.

Problem: x(4,2048,1024), Wq(8,1024,128), Wk/Wv(1024,128), Wo(1024,1024) fp32.
out = softmax(Q K^T / sqrt(128)) V -> concat heads -> @ Wo.

Sharding: (batch, query-half) across 8 cores — core c handles batch c//2,
query rows [(c%2)*1024, (c%2)*1024+1024). K/V cover the full sequence of the
batch, so each core computes them locally from its x slice; no collectives.

Numerics: scores have std ~1024 and softmax is near-one-hot, so the
x->Q/K->scores chain needs ~fp32 precision. bf16 matmuls with hi/lo split
operands ("split3": Ah*Bh + Ah*Bl + Al*Bh, fp32 PSUM accumulation) give
~5e-6 relative matmul error at 3 cycles/row (native fp32 is 4). The x and
weight splits are precomputed on the host. V/ctx/Wo paths are plain bf16.

Layouts (partition dim first):
  xT (E,S) host-transposed; K^T (O,S) = sum_e Wk[e].T-stationary @ xT[e];
  Q_h^T (O,Sq) likewise (Wq pre-scaled by 1/sqrt(O) on host);
  scores tile (128q, 2048s) = Q^T-slice-stationary @ K^T-moving, fp32 PSUM,
  bank-chunk-major so each 512-col bank finishes early;
  softmax per q-row: per-bank DVE reduce_max -> combine(negate) -> per-bank
  ACT exp(bias=-max, accum_out=den chunk) -> den sum -> 1/den -> DVE scale;
  P transposed via the DMA xbar (dma_start_transpose on the two HWDGE
  queues) — one [128,2048]->[128,16,128] instruction per q-tile, keeping
  the PE free for real matmuls (the xbar runs ~2.6us/tile-transpose
  aggregate, overlapped under the per-head matmul time);
  ctx^T (O,Sq) = V-stationary @ P^T-moving; out (Sq,E) = ctx-slices-stationary
  @ Wo-moving (natural output layout), final head's ctx interleaved with the
  out projection so the PE tail stays dense.
"""
import numpy as np
import ml_dtypes

B, S, E, H, O = 4, 2048, 1024, 8, 128
SQ = S // 2          # query rows per core
NCORES = 8
ET = E // 128        # 8 e-tiles
ST = S // 128        # 16 s-tiles
QT = SQ // 128       # 8 q-tiles
NB = S // 512        # 4 score banks per q-tile
EC = E // 512        # 2 out-proj column chunks

_compiled = None     # cache so repeated kernel() calls skip rebuild


def _build():
    import concourse.bass as bass
    import concourse.mybir as mybir
    import concourse.tile as tile
    from concourse import bacc

    F32 = mybir.dt.float32
    BF16 = mybir.dt.bfloat16
    PS = bass.MemorySpace.PSUM
    EXP = mybir.ActivationFunctionType.Exp

    nc = bacc.Bacc("TRN2", target_bir_lowering=False, debug=False,
                   enable_asserts=True)

    # xkv columns are pre-permuted per core so its query half is always
    # columns [0, SQ) — attention is permutation-invariant over the key axis,
    # so the same NEFF slices queries identically on every core.
    d_xkvh = nc.dram_tensor("xkvh", (E, S), BF16, kind="ExternalInput").ap()
    d_xkvl = nc.dram_tensor("xkvl", (E, S), BF16, kind="ExternalInput").ap()
    d_wqh = nc.dram_tensor("wqh", (H, E, O), BF16, kind="ExternalInput").ap()
    d_wql = nc.dram_tensor("wql", (H, E, O), BF16, kind="ExternalInput").ap()
    d_wkh = nc.dram_tensor("wkh", (E, O), BF16, kind="ExternalInput").ap()
    d_wkl = nc.dram_tensor("wkl", (E, O), BF16, kind="ExternalInput").ap()
    d_wvh = nc.dram_tensor("wvh", (E, O), BF16, kind="ExternalInput").ap()
    d_woh = nc.dram_tensor("woh", (H * O, E), BF16, kind="ExternalInput").ap()
    d_out = nc.dram_tensor("out", (SQ, E), F32, kind="ExternalOutput").ap()

    with tile.TileContext(nc) as tc:
        with (
            tc.tile_pool(name="persist", bufs=1) as persist,
            tc.tile_pool(name="tiny", bufs=24) as tiny,
        ):
            wo_sb = persist.tile([128, H, E], BF16, tag="wo")

            kth = persist.tile([128, S], BF16, tag="kth")
            ktl = persist.tile([128, S], BF16, tag="ktl")
            qth = persist.tile([128, H, SQ], BF16, tag="qth")
            qtl = persist.tile([128, H, SQ], BF16, tag="qtl")
            v_sb = persist.tile([128, ST, O], BF16, tag="v")

            # ---------------- prologue: K^T, V, Q^T projections ----------
            with tc.tile_pool(name="xp", bufs=1) as xp:
                wkh = xp.tile([128, ET, O], BF16, tag="wkh")
                wkl = xp.tile([128, ET, O], BF16, tag="wkl")
                xkvh = xp.tile([128, ET, S], BF16, tag="xkvh")
                xkvl = xp.tile([128, ET, S], BF16, tag="xkvl")
                wqh = xp.tile([128, H, ET, O], BF16, tag="wqh")
                wql = xp.tile([128, H, ET, O], BF16, tag="wql")
                wvh = xp.tile([128, ET, O], BF16, tag="wvh")

                # DMA order = consumption order of the Q phase, striped over
                # the three issuing queues so the first matmul's operands
                # (wqh[h0] + xkvh e0 query-half) land within a few us:
                #   sync   : wqh h0, xkvh [0:SQ] by e, wkh, xkvh [SQ:S]
                #   scalar : wql h0, xkvl [0:SQ] by e, wkl+wvh, xkvl [SQ:S], wo
                #   gpsimd : wq heads 1..7 (consumed at ~10us/head)
                nc.sync.dma_start(
                    wqh[:, 0, :, :], d_wqh[0].rearrange("(t p) o -> p t o", p=128))
                nc.scalar.dma_start(
                    wql[:, 0, :, :], d_wql[0].rearrange("(t p) o -> p t o", p=128))
                for e in range(ET):
                    nc.sync.dma_start(
                        xkvh[:, e, 0:SQ], d_xkvh[e * 128:(e + 1) * 128, 0:SQ])
                    nc.scalar.dma_start(
                        xkvl[:, e, 0:SQ], d_xkvl[e * 128:(e + 1) * 128, 0:SQ])
                for h in range(1, H):
                    nc.gpsimd.dma_start(
                        wqh[:, h, :, :],
                        d_wqh[h].rearrange("(t p) o -> p t o", p=128))
                    nc.gpsimd.dma_start(
                        wql[:, h, :, :],
                        d_wql[h].rearrange("(t p) o -> p t o", p=128))
                nc.sync.dma_start(wkh[:], d_wkh.rearrange("(t p) o -> p t o", p=128))
                nc.scalar.dma_start(wkl[:], d_wkl.rearrange("(t p) o -> p t o", p=128))
                nc.scalar.dma_start(wvh[:], d_wvh.rearrange("(t p) o -> p t o", p=128))
                for e in range(ET):
                    nc.sync.dma_start(
                        xkvh[:, e, SQ:S], d_xkvh[e * 128:(e + 1) * 128, SQ:S])
                    nc.scalar.dma_start(
                        xkvl[:, e, SQ:S], d_xkvl[e * 128:(e + 1) * 128, SQ:S])
                nc.gpsimd.dma_start(
                    wo_sb[:], d_woh.rearrange("(h p) e -> p h e", p=128))

                # Q^T per head first (needs only the query-half columns)
                with tc.tile_pool(name="qp", bufs=3, space=PS) as qp:
                    for h in range(H):
                        q_ps = qp.tile([128, SQ], F32, tag="qtps")
                        for e in range(ET):
                            for ti, (w, xx) in enumerate(
                                ((wqh, xkvh), (wqh, xkvl), (wql, xkvh))
                            ):
                                for c in range(SQ // 512):
                                    nc.tensor.matmul(
                                        q_ps[:, c * 512:(c + 1) * 512],
                                        w[:, h, e, :],
                                        xx[:, e, c * 512:(c + 1) * 512],
                                        start=(e == 0 and ti == 0),
                                        stop=(e == ET - 1 and ti == 2),
                                    )
                        nc.scalar.copy(qth[:, h, :], q_ps[:])
                        nc.vector.tensor_sub(qtl[:, h, :], q_ps[:], qth[:, h, :])

                # K^T and V^T share one PSUM scope (4+4 banks) so their
                # matmuls interleave and neither phase-transition stalls PE
                with (
                    tc.tile_pool(name="ktp", bufs=1, space=PS) as ktp,
                    tc.tile_pool(name="vtp", bufs=1, space=PS) as vtp,
                ):
                    kt_ps = ktp.tile([128, S], F32, tag="kt")
                    vt_ps = vtp.tile([128, S], F32, tag="vt")
                    for e in range(ET):
                        for ti, (w, xx) in enumerate(
                            ((wkh, xkvh), (wkh, xkvl), (wkl, xkvh))
                        ):
                            for c in range(NB):
                                nc.tensor.matmul(
                                    kt_ps[:, c * 512:(c + 1) * 512],
                                    w[:, e, :],
                                    xx[:, e, c * 512:(c + 1) * 512],
                                    start=(e == 0 and ti == 0),
                                    stop=(e == ET - 1 and ti == 2),
                                )
                        # V^T (o-part) with Wv stationary: 8 weight loads
                        for c in range(NB):
                            nc.tensor.matmul(
                                vt_ps[:, c * 512:(c + 1) * 512],
                                wvh[:, e, :],
                                xkvh[:, e, c * 512:(c + 1) * 512],
                                start=(e == 0),
                                stop=(e == ET - 1),
                            )
                    nc.scalar.copy(kth[:], kt_ps[:])
                    nc.vector.tensor_sub(ktl[:], kt_ps[:], kth[:])
                    vt_sb = xp.tile([128, S], BF16, tag="vtsb")
                    nc.scalar.copy(vt_sb[:], vt_ps[:])
                # V^T (o,s) -> V tiles (s-in-tile, st, o) on the DMA xbar
                nc.sync.dma_start_transpose(v_sb[:], vt_sb[:])

            # ---------------- main: per-head attention ------------------
            # PSUM budget (8 banks): "acc1024" 2-bank tiles x4 bufs shared by
            # score-halves, ctx and out accumulators.  Score halves cycle
            # through the free slots so the next q-tile's matmuls never wait
            # on this one's softmax.  P^T runs on the DMA xbar, not PE.
            with (
                tc.tile_pool(name="p_pool", bufs=4) as p_pool,
                tc.tile_pool(name="pt_pool", bufs=2) as pt_pool,
                tc.tile_pool(name="ctx_pool", bufs=H) as ctx_pool,
                tc.tile_pool(name="acc_ps", bufs=4, space=PS) as acc_psp,
                tc.tile_pool(name="o_sb", bufs=2) as o_sbp,
            ):
                HS = S // 2  # 1024-wide score half

                MIN = mybir.AluOpType.min
                ctxs = []

                def emit_ctx_half(state, qc):
                    # ctx^T (o-part, q-free) accumulated over s-tiles; lagged
                    # into the next head's score phase as PE filler, one
                    # 512-wide half-burst at a time to limit the disruption
                    pt_h = state["pt"]
                    ctx_h = state["ctx"]
                    if state["ct"] is None:
                        ct_ps = acc_psp.tile([128, SQ], F32, tag="acc1024")
                        state["ct"] = ct_ps
                    ct_ps = state["ct"]
                    for st in range(ST):
                        nc.tensor.matmul(
                            ct_ps[:, qc * 512:(qc + 1) * 512],
                            v_sb[:, st, :],
                            pt_h[:, st, qc * 512:(qc + 1) * 512],
                            start=(st == 0),
                            stop=(st == ST - 1),
                        )
                    nc.scalar.copy(
                        ctx_h[:, qc * 512:(qc + 1) * 512],
                        ct_ps[:, qc * 512:(qc + 1) * 512])

                pending_ctx = None
                for h in range(H):
                    pt_h = pt_pool.tile([128, ST, SQ], BF16, tag="pt")
                    for qt in range(QT):
                        # flash-style: each half gets a LOCAL max + exp so its
                        # PSUM slot frees without waiting for the other half;
                        # tiny per-partition factors fix up the normalization.
                        nm2 = tiny.tile([128, 2], F32, tag="nm2")
                        den2 = tiny.tile([128, 2], F32, tag="den2")
                        p_qt = p_pool.tile([128, S], BF16, tag="p")
                        for sh in range(2):
                            s_ps = acc_psp.tile([128, HS], F32, tag="acc1024")
                            for ti, (qq, kk) in enumerate(
                                ((qth, kth), (qth, ktl), (qtl, kth))
                            ):
                                for c in range(2):
                                    nc.tensor.matmul(
                                        s_ps[:, c * 512:(c + 1) * 512],
                                        qq[:, h, qt * 128:(qt + 1) * 128],
                                        kk[:, sh * HS + c * 512:
                                           sh * HS + (c + 1) * 512],
                                        start=(ti == 0),
                                        stop=(ti == 2),
                                    )
                            nc.vector.reduce_max(
                                out=nm2[:, sh:sh + 1], in_=s_ps[:],
                                axis=mybir.AxisListType.X, negate=True,
                            )
                            nc.scalar.activation(
                                p_qt[:, sh * HS:(sh + 1) * HS],
                                s_ps[:],
                                EXP, bias=nm2[:, sh:sh + 1], scale=1.0,
                                accum_out=den2[:, sh:sh + 1],
                            )
                        # fixup: p *= exp(m_sh - m_glob) / den_glob, all [128,·]
                        nmg = tiny.tile([128, 1], F32, tag="nmg")
                        nc.vector.tensor_reduce(
                            out=nmg[:], in_=nm2[:],
                            axis=mybir.AxisListType.X, op=MIN,
                        )
                        f2 = tiny.tile([128, 2], F32, tag="f2")
                        nc.scalar.activation(
                            f2[:], nm2[:], EXP, bias=nmg[:], scale=-1.0)
                        t2 = tiny.tile([128, 2], F32, tag="t2")
                        nc.vector.tensor_mul(t2[:], den2[:], f2[:])
                        den = tiny.tile([128, 1], F32, tag="den")
                        nc.vector.tensor_add(den[:], t2[:, 0:1], t2[:, 1:2])
                        invden = tiny.tile([128, 1], F32, tag="invden")
                        nc.vector.reciprocal(invden[:], den[:])
                        for sh in range(2):
                            nc.vector.tensor_scalar(
                                out=p_qt[:, sh * HS:(sh + 1) * HS],
                                in0=p_qt[:, sh * HS:(sh + 1) * HS],
                                scalar1=f2[:, sh:sh + 1],
                                scalar2=invden[:],
                                op0=mybir.AluOpType.mult,
                                op1=mybir.AluOpType.mult,
                            )

                        # P^T on the DMA xbar: one [128,2048]->[128,16,128]
                        # transpose per q-tile, alternating HWDGE queues
                        dq = nc.sync if qt % 2 else nc.scalar
                        dq.dma_start_transpose(
                            pt_h[:, :, qt * 128:(qt + 1) * 128], p_qt[:])

                        if pending_ctx is not None and qt in (1, 3):
                            emit_ctx_half(pending_ctx, qt // 2)
                            if qt == 3:
                                pending_ctx = None
                    ctx_h = ctx_pool.tile([128, SQ], BF16, tag="ctx")
                    pending_ctx = {"pt": pt_h, "ct": None, "ctx": ctx_h}
                    ctxs.append(ctx_h)

                # ------- out (q-part, e-free) = sum_h ctx_h^T-slices @ Wo_h
                # final head's ctx halves interleave with the out projection
                # so the PE tail stays dense
                def emit_out(qt):
                    o_ps = acc_psp.tile([128, E], F32, tag="acc1024")
                    for h in range(H):
                        for ec in range(EC):
                            nc.tensor.matmul(
                                o_ps[:, ec * 512:(ec + 1) * 512],
                                ctxs[h][:, qt * 128:(qt + 1) * 128],
                                wo_sb[:, h, ec * 512:(ec + 1) * 512],
                                start=(h == 0),
                                stop=(h == H - 1),
                            )
                    o_sb = o_sbp.tile([128, E], F32, tag="osb")
                    nc.scalar.copy(o_sb[:], o_ps[:])
                    dq = nc.gpsimd if qt % 2 else nc.sync
                    dq.dma_start(d_out[qt * 128:(qt + 1) * 128, :], o_sb[:])

                emit_ctx_half(pending_ctx, 0)
                for qt in range(0, 4):
                    emit_out(qt)
                emit_ctx_half(pending_ctx, 1)
                for qt in range(4, QT):
                    emit_out(qt)

    nc.compile()
    return nc


def _split(a):
    """fp32 -> (hi, lo) bf16 pair with hi + lo ~= a."""
    hi = a.astype(ml_dtypes.bfloat16)
    lo = (a - hi.astype(np.float32)).astype(ml_dtypes.bfloat16)
    return hi, lo


def kernel(x, Wq, Wk, Wv, Wo):
    global _compiled
    from concourse.bass_utils import run_bass_kernel_spmd

    x = np.asarray(x, dtype=np.float32)
    Wq = np.asarray(Wq, dtype=np.float32)
    Wk = np.asarray(Wk, dtype=np.float32)
    Wv = np.asarray(Wv, dtype=np.float32)
    Wo = np.asarray(Wo, dtype=np.float32)

    if _compiled is None:
        _compiled = _build()
    nc = _compiled

    scale = np.float32(1.0 / np.sqrt(O))
    wqh, wql = _split(Wq.astype(np.float32) * scale)
    wkh, wkl = _split(Wk.astype(np.float32))
    wvh = Wv.astype(ml_dtypes.bfloat16)
    woh = Wo.astype(ml_dtypes.bfloat16)

    in_maps = []
    xsplits = {}
    for b in range(B):
        xsplits[b] = _split(np.ascontiguousarray(x[b].T))  # (E, S) fp32
    for c in range(NCORES):
        b, half = divmod(c, 2)
        xh, xl = xsplits[b]
        if half == 0:
            ph, pl = xh, xl
        else:
            # rotate so this core's query half occupies columns [0, SQ);
            # attention is permutation-invariant over the key/value axis
            ph = np.ascontiguousarray(np.roll(xh, SQ, axis=1))
            pl = np.ascontiguousarray(np.roll(xl, SQ, axis=1))
        in_maps.append({
            "xkvh": ph, "xkvl": pl,
            "wqh": wqh, "wql": wql,
            "wkh": wkh, "wkl": wkl, "wvh": wvh, "woh": woh,
        })

    res = run_bass_kernel_spmd(nc, in_maps, core_ids=list(range(NCORES)))

    out = np.empty((B, S, E), dtype=np.float32)
    for c in range(NCORES):
        b, half = divmod(c, 2)
        out[b, half * SQ:(half + 1) * SQ, :] = res.results[c]["out"]
    return out


# revision 4
# speedup vs baseline: 1.0956x; 1.0956x over previous
"""Multi-head self-attention on 8 TRN2 NeuronCores.

Problem: x(4,2048,1024), Wq(8,1024,128), Wk/Wv(1024,128), Wo(1024,1024) fp32.
out = softmax(Q K^T / sqrt(128)) V -> concat heads -> @ Wo.

Sharding: (batch, query-half) across 8 cores — core c handles batch c//2,
query rows [(c%2)*1024, (c%2)*1024+1024). K/V cover the full sequence of the
batch, so each core computes them locally from its x slice; no collectives.

Numerics: scores have std ~1024 and softmax is near-one-hot, so the
x->Q/K->scores chain needs ~fp32 precision. bf16 matmuls with hi/lo split
operands ("split3": Ah*Bh + Ah*Bl + Al*Bh, fp32 PSUM accumulation) give
~5e-6 relative matmul error at 3 cycles/row (native fp32 is 4). The x and
weight splits are precomputed on the host. V/ctx/Wo paths are plain bf16.

Layouts (partition dim first):
  xT (E,S) host-transposed; K^T (O,S) = sum_e Wk[e].T-stationary @ xT[e];
  Q_h^T (O,Sq) likewise (Wq pre-scaled by 1/sqrt(O) on host);
  scores tile (128q, 2048s) = Q^T-slice-stationary @ K^T-moving, fp32 PSUM,
  bank-chunk-major so each 512-col bank finishes early;
  softmax per q-row: per-bank DVE reduce_max -> combine(negate) -> per-bank
  ACT exp(bias=-max, accum_out=den chunk) -> den sum -> 1/den -> DVE scale;
  P transposed via the DMA xbar (dma_start_transpose on the two HWDGE
  queues) — one [128,2048]->[128,16,128] instruction per q-tile, keeping
  the PE free for real matmuls (the xbar runs ~2.6us/tile-transpose
  aggregate, overlapped under the per-head matmul time);
  ctx^T (O,Sq) = V-stationary @ P^T-moving; out (Sq,E) = ctx-slices-stationary
  @ Wo-moving (natural output layout), final head's ctx interleaved with the
  out projection so the PE tail stays dense.
"""
import numpy as np
import ml_dtypes

B, S, E, H, O = 4, 2048, 1024, 8, 128
SQ = S // 2          # query rows per core
NCORES = 8
ET = E // 128        # 8 e-tiles
ST = S // 128        # 16 s-tiles
QT = SQ // 128       # 8 q-tiles
NB = S // 512        # 4 score banks per q-tile
EC = E // 512        # 2 out-proj column chunks

_compiled = None     # cache so repeated kernel() calls skip rebuild


def _build():
    import concourse.bass as bass
    import concourse.mybir as mybir
    import concourse.tile as tile
    from concourse import bacc

    F32 = mybir.dt.float32
    BF16 = mybir.dt.bfloat16
    PS = bass.MemorySpace.PSUM
    EXP = mybir.ActivationFunctionType.Exp

    nc = bacc.Bacc("TRN2", target_bir_lowering=False, debug=False,
                   enable_asserts=True)

    # xkv columns are pre-permuted per core so its query half is always
    # columns [0, SQ) — attention is permutation-invariant over the key axis,
    # so the same NEFF slices queries identically on every core.
    d_xkvh = nc.dram_tensor("xkvh", (E, S), BF16, kind="ExternalInput").ap()
    d_xkvl = nc.dram_tensor("xkvl", (E, S), BF16, kind="ExternalInput").ap()
    d_wqh = nc.dram_tensor("wqh", (H, E, O), BF16, kind="ExternalInput").ap()
    d_wql = nc.dram_tensor("wql", (H, E, O), BF16, kind="ExternalInput").ap()
    d_wkh = nc.dram_tensor("wkh", (E, O), BF16, kind="ExternalInput").ap()
    d_wkl = nc.dram_tensor("wkl", (E, O), BF16, kind="ExternalInput").ap()
    d_wvh = nc.dram_tensor("wvh", (E, O), BF16, kind="ExternalInput").ap()
    d_woh = nc.dram_tensor("woh", (H * O, E), BF16, kind="ExternalInput").ap()
    d_out = nc.dram_tensor("out", (SQ, E), F32, kind="ExternalOutput").ap()

    with tile.TileContext(nc) as tc:
        with (
            tc.tile_pool(name="persist", bufs=1) as persist,
            tc.tile_pool(name="tiny", bufs=24) as tiny,
        ):
            wo_sb = persist.tile([128, H, E], BF16, tag="wo")

            kth = persist.tile([128, S], BF16, tag="kth")
            ktl = persist.tile([128, S], BF16, tag="ktl")
            qth = persist.tile([128, H, SQ], BF16, tag="qth")
            qtl = persist.tile([128, H, SQ], BF16, tag="qtl")
            v_sb = persist.tile([128, ST, O], BF16, tag="v")

            # ---------------- prologue: K^T, V, Q^T projections ----------
            with tc.tile_pool(name="xp", bufs=1) as xp:
                wkh = xp.tile([128, ET, O], BF16, tag="wkh")
                wkl = xp.tile([128, ET, O], BF16, tag="wkl")
                xkvh = xp.tile([128, ET, S], BF16, tag="xkvh")
                xkvl = xp.tile([128, ET, S], BF16, tag="xkvl")
                wqh = xp.tile([128, H, ET, O], BF16, tag="wqh")
                wql = xp.tile([128, H, ET, O], BF16, tag="wql")
                wvh = xp.tile([128, ET, O], BF16, tag="wvh")

                # DMA order = consumption order of the Q phase, striped over
                # the three issuing queues so the first matmul's operands
                # (wqh[h0] + xkvh e0 query-half) land within a few us:
                #   sync   : wqh h0, xkvh [0:SQ] by e, wkh, xkvh [SQ:S]
                #   scalar : wql h0, xkvl [0:SQ] by e, wkl+wvh, xkvl [SQ:S], wo
                #   gpsimd : wq heads 1..7 (consumed at ~10us/head)
                nc.sync.dma_start(
                    wqh[:, 0, :, :], d_wqh[0].rearrange("(t p) o -> p t o", p=128))
                nc.scalar.dma_start(
                    wql[:, 0, :, :], d_wql[0].rearrange("(t p) o -> p t o", p=128))
                for e in range(ET):
                    nc.sync.dma_start(
                        xkvh[:, e, 0:SQ], d_xkvh[e * 128:(e + 1) * 128, 0:SQ])
                    nc.scalar.dma_start(
                        xkvl[:, e, 0:SQ], d_xkvl[e * 128:(e + 1) * 128, 0:SQ])
                for h in range(1, H):
                    nc.gpsimd.dma_start(
                        wqh[:, h, :, :],
                        d_wqh[h].rearrange("(t p) o -> p t o", p=128))
                    nc.gpsimd.dma_start(
                        wql[:, h, :, :],
                        d_wql[h].rearrange("(t p) o -> p t o", p=128))
                nc.sync.dma_start(wkh[:], d_wkh.rearrange("(t p) o -> p t o", p=128))
                nc.scalar.dma_start(wkl[:], d_wkl.rearrange("(t p) o -> p t o", p=128))
                nc.scalar.dma_start(wvh[:], d_wvh.rearrange("(t p) o -> p t o", p=128))
                for e in range(ET):
                    nc.sync.dma_start(
                        xkvh[:, e, SQ:S], d_xkvh[e * 128:(e + 1) * 128, SQ:S])
                    nc.scalar.dma_start(
                        xkvl[:, e, SQ:S], d_xkvl[e * 128:(e + 1) * 128, SQ:S])
                nc.gpsimd.dma_start(
                    wo_sb[:], d_woh.rearrange("(h p) e -> p h e", p=128))

                # Q^T per head first (needs only the query-half columns)
                with tc.tile_pool(name="qp", bufs=3, space=PS) as qp:
                    for h in range(H):
                        q_ps = qp.tile([128, SQ], F32, tag="qtps")
                        for e in range(ET):
                            for ti, (w, xx) in enumerate(
                                ((wqh, xkvh), (wqh, xkvl), (wql, xkvh))
                            ):
                                for c in range(SQ // 512):
                                    nc.tensor.matmul(
                                        q_ps[:, c * 512:(c + 1) * 512],
                                        w[:, h, e, :],
                                        xx[:, e, c * 512:(c + 1) * 512],
                                        start=(e == 0 and ti == 0),
                                        stop=(e == ET - 1 and ti == 2),
                                    )
                        nc.scalar.copy(qth[:, h, :], q_ps[:])
                        nc.vector.tensor_sub(qtl[:, h, :], q_ps[:], qth[:, h, :])

                # K^T and V^T share one PSUM scope (4+4 banks) so their
                # matmuls interleave and neither phase-transition stalls PE
                with (
                    tc.tile_pool(name="ktp", bufs=1, space=PS) as ktp,
                    tc.tile_pool(name="vtp", bufs=1, space=PS) as vtp,
                ):
                    kt_ps = ktp.tile([128, S], F32, tag="kt")
                    vt_ps = vtp.tile([128, S], F32, tag="vt")
                    for e in range(ET):
                        for ti, (w, xx) in enumerate(
                            ((wkh, xkvh), (wkh, xkvl), (wkl, xkvh))
                        ):
                            for c in range(NB):
                                nc.tensor.matmul(
                                    kt_ps[:, c * 512:(c + 1) * 512],
                                    w[:, e, :],
                                    xx[:, e, c * 512:(c + 1) * 512],
                                    start=(e == 0 and ti == 0),
                                    stop=(e == ET - 1 and ti == 2),
                                )
                        # V^T (o-part) with Wv stationary: 8 weight loads
                        for c in range(NB):
                            nc.tensor.matmul(
                                vt_ps[:, c * 512:(c + 1) * 512],
                                wvh[:, e, :],
                                xkvh[:, e, c * 512:(c + 1) * 512],
                                start=(e == 0),
                                stop=(e == ET - 1),
                            )
                    nc.scalar.copy(kth[:], kt_ps[:])
                    nc.vector.tensor_sub(ktl[:], kt_ps[:], kth[:])
                    vt_sb = xp.tile([128, S], BF16, tag="vtsb")
                    nc.scalar.copy(vt_sb[:], vt_ps[:])
                # V^T (o,s) -> V tiles (s-in-tile, st, o) on the DMA xbar
                nc.sync.dma_start_transpose(v_sb[:], vt_sb[:])

            # ---------------- main: per-head attention ------------------
            # PSUM budget (8 banks): "acc1024" 2-bank tiles x4 bufs shared by
            # score-halves, ctx and out accumulators.  Score halves cycle
            # through the free slots so the next q-tile's matmuls never wait
            # on this one's softmax.  P^T runs on the DMA xbar, not PE.
            with (
                tc.tile_pool(name="p_pool", bufs=4) as p_pool,
                tc.tile_pool(name="pt_pool", bufs=2) as pt_pool,
                tc.tile_pool(name="ctx_pool", bufs=H) as ctx_pool,
                tc.tile_pool(name="acc_ps", bufs=4, space=PS) as acc_psp,
                tc.tile_pool(name="o_sb", bufs=2) as o_sbp,
            ):
                HS = S // 2  # 1024-wide score half

                MIN = mybir.AluOpType.min
                ctxs = []

                def emit_ctx_half(state, qc):
                    # ctx^T (o-part, q-free) accumulated over s-tiles; lagged
                    # into the next head's score phase as PE filler, one
                    # 512-wide half-burst at a time to limit the disruption
                    pt_h = state["pt"]
                    ctx_h = state["ctx"]
                    if state["ct"] is None:
                        ct_ps = acc_psp.tile([128, SQ], F32, tag="acc1024")
                        state["ct"] = ct_ps
                    ct_ps = state["ct"]
                    for st in range(ST):
                        nc.tensor.matmul(
                            ct_ps[:, qc * 512:(qc + 1) * 512],
                            v_sb[:, st, :],
                            pt_h[:, st, qc * 512:(qc + 1) * 512],
                            start=(st == 0),
                            stop=(st == ST - 1),
                        )
                    nc.scalar.copy(
                        ctx_h[:, qc * 512:(qc + 1) * 512],
                        ct_ps[:, qc * 512:(qc + 1) * 512])

                pending_ctx = None
                for h in range(H):
                    pt_h = pt_pool.tile([128, ST, SQ], BF16, tag="pt")
                    for qt in range(QT):
                        # flash-style: each half gets a LOCAL max + exp so its
                        # PSUM slot frees without waiting for the other half;
                        # tiny per-partition factors fix up the normalization.
                        nm2 = tiny.tile([128, 2], F32, tag="nm2")
                        den2 = tiny.tile([128, 2], F32, tag="den2")
                        p_qt = p_pool.tile([128, S], BF16, tag="p")
                        for sh in range(2):
                            s_ps = acc_psp.tile([128, HS], F32, tag="acc1024")
                            for ti, (qq, kk) in enumerate(
                                ((qth, kth), (qth, ktl), (qtl, kth))
                            ):
                                for c in range(2):
                                    nc.tensor.matmul(
                                        s_ps[:, c * 512:(c + 1) * 512],
                                        qq[:, h, qt * 128:(qt + 1) * 128],
                                        kk[:, sh * HS + c * 512:
                                           sh * HS + (c + 1) * 512],
                                        start=(ti == 0),
                                        stop=(ti == 2),
                                    )
                            nc.vector.reduce_max(
                                out=nm2[:, sh:sh + 1], in_=s_ps[:],
                                axis=mybir.AxisListType.X, negate=True,
                            )
                            nc.scalar.activation(
                                p_qt[:, sh * HS:(sh + 1) * HS],
                                s_ps[:],
                                EXP, bias=nm2[:, sh:sh + 1], scale=1.0,
                                accum_out=den2[:, sh:sh + 1],
                            )
                        # fixup: p *= exp(m_sh - m_glob) / den_glob, all [128,·]
                        nmg = tiny.tile([128, 1], F32, tag="nmg")
                        nc.vector.tensor_reduce(
                            out=nmg[:], in_=nm2[:],
                            axis=mybir.AxisListType.X, op=MIN,
                        )
                        f2 = tiny.tile([128, 2], F32, tag="f2")
                        nc.scalar.activation(
                            f2[:], nm2[:], EXP, bias=nmg[:], scale=-1.0)
                        t2 = tiny.tile([128, 2], F32, tag="t2")
                        nc.vector.tensor_mul(t2[:], den2[:], f2[:])
                        den = tiny.tile([128, 1], F32, tag="den")
                        nc.vector.tensor_add(den[:], t2[:, 0:1], t2[:, 1:2])
                        invden = tiny.tile([128, 1], F32, tag="invden")
                        nc.vector.reciprocal(invden[:], den[:])
                        for sh in range(2):
                            nc.vector.tensor_scalar(
                                out=p_qt[:, sh * HS:(sh + 1) * HS],
                                in0=p_qt[:, sh * HS:(sh + 1) * HS],
                                scalar1=f2[:, sh:sh + 1],
                                scalar2=invden[:],
                                op0=mybir.AluOpType.mult,
                                op1=mybir.AluOpType.mult,
                            )

                        # P^T on the DMA xbar: one [128,2048]->[128,16,128]
                        # transpose per q-tile.  DMA_TRANSPOSE occupies the
                        # issuing sequencer for the whole transfer, so keep
                        # them all on sync — the scalar queue must stay free
                        # for the latency-critical exp stream.
                        nc.sync.dma_start_transpose(
                            pt_h[:, :, qt * 128:(qt + 1) * 128], p_qt[:])

                        if pending_ctx is not None and qt in (1, 3):
                            emit_ctx_half(pending_ctx, qt // 2)
                            if qt == 3:
                                pending_ctx = None
                    ctx_h = ctx_pool.tile([128, SQ], BF16, tag="ctx")
                    pending_ctx = {"pt": pt_h, "ct": None, "ctx": ctx_h}
                    ctxs.append(ctx_h)

                # ------- out (q-part, e-free) = sum_h ctx_h^T-slices @ Wo_h
                # final head's ctx halves interleave with the out projection
                # so the PE tail stays dense
                def emit_out(qt):
                    o_ps = acc_psp.tile([128, E], F32, tag="acc1024")
                    for h in range(H):
                        for ec in range(EC):
                            nc.tensor.matmul(
                                o_ps[:, ec * 512:(ec + 1) * 512],
                                ctxs[h][:, qt * 128:(qt + 1) * 128],
                                wo_sb[:, h, ec * 512:(ec + 1) * 512],
                                start=(h == 0),
                                stop=(h == H - 1),
                            )
                    o_sb = o_sbp.tile([128, E], F32, tag="osb")
                    nc.scalar.copy(o_sb[:], o_ps[:])
                    nc.gpsimd.dma_start(
                        d_out[qt * 128:(qt + 1) * 128, :], o_sb[:])

                emit_ctx_half(pending_ctx, 0)
                for qt in range(0, 4):
                    emit_out(qt)
                emit_ctx_half(pending_ctx, 1)
                for qt in range(4, QT):
                    emit_out(qt)

    nc.compile()
    return nc


def _split(a):
    """fp32 -> (hi, lo) bf16 pair with hi + lo ~= a."""
    hi = a.astype(ml_dtypes.bfloat16)
    lo = (a - hi.astype(np.float32)).astype(ml_dtypes.bfloat16)
    return hi, lo


def kernel(x, Wq, Wk, Wv, Wo):
    global _compiled
    from concourse.bass_utils import run_bass_kernel_spmd

    x = np.asarray(x, dtype=np.float32)
    Wq = np.asarray(Wq, dtype=np.float32)
    Wk = np.asarray(Wk, dtype=np.float32)
    Wv = np.asarray(Wv, dtype=np.float32)
    Wo = np.asarray(Wo, dtype=np.float32)

    if _compiled is None:
        _compiled = _build()
    nc = _compiled

    scale = np.float32(1.0 / np.sqrt(O))
    wqh, wql = _split(Wq.astype(np.float32) * scale)
    wkh, wkl = _split(Wk.astype(np.float32))
    wvh = Wv.astype(ml_dtypes.bfloat16)
    woh = Wo.astype(ml_dtypes.bfloat16)

    in_maps = []
    xsplits = {}
    for b in range(B):
        xsplits[b] = _split(np.ascontiguousarray(x[b].T))  # (E, S) fp32
    for c in range(NCORES):
        b, half = divmod(c, 2)
        xh, xl = xsplits[b]
        if half == 0:
            ph, pl = xh, xl
        else:
            # rotate so this core's query half occupies columns [0, SQ);
            # attention is permutation-invariant over the key/value axis
            ph = np.ascontiguousarray(np.roll(xh, SQ, axis=1))
            pl = np.ascontiguousarray(np.roll(xl, SQ, axis=1))
        in_maps.append({
            "xkvh": ph, "xkvl": pl,
            "wqh": wqh, "wql": wql,
            "wkh": wkh, "wkl": wkl, "wvh": wvh, "woh": woh,
        })

    res = run_bass_kernel_spmd(nc, in_maps, core_ids=list(range(NCORES)))

    out = np.empty((B, S, E), dtype=np.float32)
    for c in range(NCORES):
        b, half = divmod(c, 2)
        out[b, half * SQ:(half + 1) * SQ, :] = res.results[c]["out"]
    return out
